# revision 24
# baseline (speedup 1.0000x reference)
"""Causal self-attention (B=2, S=2048, D=1024, H=16, hd=64) on 8 TRN2 NeuronCores.

Sharding: batch x head-group. Core c handles batch c//4 and heads
4*(c%4) .. 4*(c%4)+3. Each core computes its 4 heads' attention plus the
partial output projection; the host sums the 4 partial projections per batch.

v3 (vs the 189us v2): head-pair-interleaved schedule so the Act engine's
~82us of exp work starts at ~30us instead of ~70us.
  - P0a: q/k projection for pair 0 only, k-outer over both mts (8 PSUM
    banks), paced by the x-tile DMA arrivals; first x tile split so the
    first matmul starts ~5us earlier.
  - P0b: pair-0 qpre copies (split Act/DVE) + pair-0 RoPE flushes.
  - attention hp=0 streams run with a 40-closure PE filler list: v for
    all 4 heads (N=256 chains), pair-1 q/k chains (split in 4-matmul
    halves through a single 1-bank PSUM ring), pair-1 RoPE. Ordered so
    each closure lands just before its consumer.
  - attention hp=1 streams use the deferred projection units as filler
    (popped every jt, not every other).
  - AV matmuls and probs reads start at column c0 on diagonal key-tiles
    (the masked-out query range is never computed, written, or read --
    kills the zero-memsets of v2).
  - drain chain: reciprocal input is the only PSUM->SBUF staging copy;
    the normalize multiply reads the AV accumulator straight from PSUM.
  - a dummy matmul keyed on the final drain keeps the PE HAM clock-gate
    warm across the tail.
"""

import sys

try:
    import concourse.bass  # noqa: F401
except ImportError:
    sys.path.insert(0, "/opt/trn_rl_repo")

import numpy as np
import concourse.bacc as bacc
import concourse.mybir as mybir
from concourse.tile import TileContext
from concourse.bass_utils import run_bass_kernel_spmd

F32 = mybir.dt.float32
F16 = mybir.dt.float16

B, S, D = 2, 2048, 1024
H, HD = 16, 64
HEADS_PER_CORE = 4
N_CORES = 8
ROPE_BASE = 10000.0
SCALE = HD ** -0.5

KT = D // 128          # 8  contraction tiles for the QKV projection
ST = S // 128          # 16 sequence tiles of 128
NC_CH = S // 512       # 4  sequence chunks of 512
WF = 3 * HEADS_PER_CORE * HD   # 768 projection features per core
VOFF = 2 * HEADS_PER_CORE * HD # 512 column offset of the v block in w

CHUNK_ORDER = [1, 0, 3, 2]


def _build_program():
    nc = bacc.Bacc("TRN2", target_bir_lowering=False, debug=False,
                   num_devices=N_CORES)

    xT = nc.dram_tensor("xT", [128, KT * S], F16, kind="ExternalInput")
    w = nc.dram_tensor("w", [128, KT * WF], F16, kind="ExternalInput")
    wo = nc.dram_tensor("wo", [128, 2 * D], F16, kind="ExternalInput")
    cosT = nc.dram_tensor("cosT", [128, S], F16, kind="ExternalInput")
    sinT = nc.dram_tensor("sinT", [128, S], F16, kind="ExternalInput")
    rmatT = nc.dram_tensor("rmatT", [128, 128], F16, kind="ExternalInput")
    mask2 = nc.dram_tensor("mask2", [128, 256], F16, kind="ExternalInput")
    y = nc.dram_tensor("y", [S, D], F16, kind="ExternalOutput")

    with TileContext(nc) as tc:
        with (
            tc.tile_pool(name="const", bufs=1) as constp,
            tc.tile_pool(name="acts", bufs=1) as actsp,
        ):
            w_sb = constp.tile([128, KT * WF], F16)
            wo_sb = constp.tile([128, 2 * D], F16)
            cos_sb = constp.tile([128, S], F16)
            sin_sb = constp.tile([128, S], F16)
            rmat_sb = constp.tile([128, 128], F16)
            mask_sb = constp.tile([128, 256], F16)
            warm_sb = constp.tile([128, 8], F16)
            warm2_sb = constp.tile([128, 8], F16)

            # gpsimd ISA library preload: a dummy broadcast at t=0 so the
            # ~7us lazy lib load overlaps the input DMAs. NOTE: the dst AP
            # must sit at base partition 0 -- partition_broadcast with a
            # non-zero base partition folds the partition offset into the
            # byte address and scribbles over a neighboring tile.
            nc.vector.memset(warm_sb[0:1, :], 1.0)
            nc.gpsimd.partition_broadcast(warm2_sb[0:64, :], warm_sb[0:1, :])

            xT_sb = actsp.tile([128, KT * S], F16)
            # activations produced by the QKV work, consumed by attention
            qT_sb = actsp.tile([128, 2 * S], F16)   # head pairs 0|1
            kT_sb = actsp.tile([128, 2 * S], F16)
            v_sb = actsp.tile([128, ST * 260], F16) # 16 seq tiles x 4x65
            # per-chunk normalized attention output [d(2 heads), hp*512+q]
            outTh = [[actsp.tile([128, 512], F16, name=f"outT{_c}_{_h}")
                      for _h in range(2)] for _c in range(NC_CH)]

            # input DMAs: x/w alternate on the two HWDGE queues; first x
            # tile split so the first warmup matmul only waits on 128KB.
            for k in range(KT):
                qa, qb = (nc.sync, nc.scalar) if k % 2 == 0 else (nc.scalar, nc.sync)
                if k == 0:
                    qa.dma_start(xT_sb[:, 0:512], xT[:, 0:512])
                    qb.dma_start(w_sb[:, 0:WF], w[:, 0:WF])
                    qa.dma_start(xT_sb[:, 512:S], xT[:, 512:S])
                    continue
                qa.dma_start(
                    xT_sb[:, k * S:(k + 1) * S], xT[:, k * S:(k + 1) * S])
                qb.dma_start(
                    w_sb[:, k * WF:(k + 1) * WF], w[:, k * WF:(k + 1) * WF])
            # bulky late-needed constants on gpsimd SWDGE (queued behind
            # the lib load); mask early -- first diagonal tile needs it
            # ~4us into attention.
            nc.gpsimd.dma_start(rmat_sb[:], rmatT[:])
            nc.gpsimd.dma_start(mask_sb[:], mask2[:])
            nc.gpsimd.dma_start(cos_sb[:], cosT[:])
            nc.gpsimd.dma_start(sin_sb[:], sinT[:])
            nc.gpsimd.dma_start(wo_sb[:], wo[:])

            # ones columns of the v blocks (col 64 of each 65-block)
            ones_cols = v_sb[:, 0:ST * 260].rearrange(
                "p (b c) -> p b c", c=65)[:, :, 64:65]
            nc.vector.memset(ones_cols, 1.0)

            rope_entries = {}   # (mt, n) -> (dest, doff, n, qpre tile)

            def flush_rope(key, rotpool, rottag, ropepool, gps=False):
                """dest = qpre*cos + rot(qpre)*sin. With gps=True the two
                SBUF-only elementwise ops run on gpsimd (idle during
                attention) so only the PSUM-reading t2 multiply costs DVE
                time."""
                dest, doff, n, qpre = rope_entries.pop(key)
                rot = rotpool.tile([128, 512], F32, name="rot", tag=rottag)
                nc.tensor.matmul(rot[:], rmat_sb[:], qpre[:],
                                 start=True, stop=True)
                t1 = ropepool.tile([128, 512], F16, name="t1", tag="t1")
                t2 = ropepool.tile([128, 512], F16, name="t2", tag="t2")
                nc.vector.tensor_mul(
                    t1[:], qpre[:], cos_sb[:, n * 512:(n + 1) * 512])
                nc.vector.tensor_mul(
                    t2[:], rot[:], sin_sb[:, n * 512:(n + 1) * 512])
                nc.vector.tensor_add(
                    dest[:, doff + n * 512: doff + (n + 1) * 512],
                    t1[:], t2[:])

            # ---------------- P0a: pair-0 q/k, k-outer ----------------
            with tc.tile_pool(name="wps", bufs=1, space="PSUM") as wps:
                pts = {mt: [wps.tile([128, 512], F32,
                                     name=f"wp{mt}_{n}", tag=f"wp{mt}_{n}")
                            for n in range(NC_CH)] for mt in (0, 2)}
                for k in range(KT):
                    for mt in (0, 2):
                        for n in range(NC_CH):
                            nc.tensor.matmul(
                                pts[mt][n][:],
                                w_sb[:, k * WF + mt * 128: k * WF + (mt + 1) * 128],
                                xT_sb[:, k * S + n * 512: k * S + (n + 1) * 512],
                                start=(k == 0), stop=(k == KT - 1))
                for mt in (0, 2):
                    dest = qT_sb if mt < 2 else kT_sb
                    for n in range(NC_CH):
                        qpre = actsp.tile([128, 512], F16,
                                          name=f"q{mt}_{n}")
                        if n % 2 == 0:
                            nc.scalar.copy(qpre[:], pts[mt][n][:])
                        else:
                            nc.vector.tensor_copy(qpre[:], pts[mt][n][:])
                        rope_entries[(mt, n)] = (dest, 0, n, qpre)

            # ------- P0b: only the two RoPE flushes the first ------
            # attention stream needs right away; the rest are fill
            # closures inside the hp=0 phase (their ~1.5us/flush DVE
            # chain would otherwise serialize here with the PE idle).
            with (
                tc.tile_pool(name="rotb", bufs=2, space="PSUM") as rotb,
                tc.tile_pool(name="ropet0", bufs=2) as ropetp0,
            ):
                for key in [(0, 1), (2, 0)]:
                    flush_rope(key, rotb, "rot0", ropetp0)

            # ---------------- attention + interleaved pair-1 QKV ----------
            with (
                tc.tile_pool(name="scps", bufs=2, space="PSUM") as scps,
                tc.tile_pool(name="avps", bufs=1, space="PSUM") as avps,
                tc.tile_pool(name="probs", bufs=7) as probsp,
                tc.tile_pool(name="rrp", bufs=2) as rrp,
                tc.tile_pool(name="binv", bufs=2) as binvp,
                tc.tile_pool(name="ysb", bufs=2) as ysbp,
            ):
                mask3 = mask_sb[:, 0:256].rearrange("p (b c) -> p b c", b=2)
                yps_ref = {}
                pending = []   # deferred projection-unit closures

                def make_units(pc, last=False):
                    ycb = {}

                    def unit(u, pc=pc, ycb=ycb):
                        yps = yps_ref["pool"]
                        if u == 0:
                            ycb["t"] = ysbp.tile([128, 4096], F16, name="ycb",
                                                 tag="ycb")
                        sti, nn = u // 2, u % 2
                        py = yps.tile([128, 512], F32, name="py", tag="py")
                        for hp2 in range(2):
                            nc.tensor.matmul(
                                py[:],
                                outTh[pc][hp2][:, sti * 128:(sti + 1) * 128],
                                wo_sb[:, hp2 * D + nn * 512: hp2 * D + (nn + 1) * 512],
                                start=(hp2 == 0), stop=(hp2 == 1))
                        ycs = ycb["t"][:, sti * 1024 + nn * 512:
                                       sti * 1024 + (nn + 1) * 512]
                        if last:
                            nc.scalar.copy(ycs, py[:])
                        else:
                            nc.vector.tensor_copy(ycs, py[:])
                        if nn == 1:
                            st = pc * 4 + sti
                            nc.sync.dma_start(
                                y[st * 128:(st + 1) * 128, :],
                                ycb["t"][:, sti * 1024:(sti + 1) * 1024])
                    return [lambda u=u: unit(u) for u in range(8)]

                def emit_av(hp, pav, jt, pp, c0, stop):
                    # c0 > 0 on diagonal key-tiles: queries < c0 get no
                    # contribution from this tile, so both the matmul N and
                    # the probs read start at c0.
                    for e in range(2):
                        h = 2 * hp + e
                        nc.tensor.matmul(
                            pav[e][0:65, c0:512],
                            v_sb[:, jt * 260 + h * 65: jt * 260 + (h + 1) * 65],
                            pp[:, e * 512 + c0:(e + 1) * 512],
                            start=(jt == 0), stop=stop)

                def drain(carry):
                    """AV-drain + denominator chain of the previous (ic, hp)
                    stream. The normalize multiply reads the AV accumulator
                    straight from PSUM; only the reciprocal input is staged
                    (custom-DVE ops cannot read PSUM)."""
                    ic, hp, pav, pipe = carry
                    while pipe:
                        jt_, pp_, c0_ = pipe.pop(0)
                        emit_av(hp, pav, jt_, pp_, c0_, stop=(not pipe))
                    dbs = []
                    for e in range(2):
                        dr = rrp.tile([1, 512], F32, name="dr", tag=f"dr{e}")
                        nc.vector.tensor_copy(dr[0:1, :], pav[e][64:65, :])
                        rr = rrp.tile([1, 512], F32, name="rr", tag=f"rr{e}")
                        nc.vector.reciprocal_approx_fast(
                            rr[0:1, :], dr[0:1, :])
                        db = binvp.tile([64, 512], F32, name="db",
                                        tag=f"db{e}")
                        nc.gpsimd.partition_broadcast(db[0:64, :], rr[0:1, :])
                        dbs.append(db)
                    for e in range(2):
                        nc.vector.tensor_mul(
                            outTh[ic][hp][64 * e:64 * (e + 1), :],
                            pav[e][0:64, :], dbs[e][0:64, :])
                    return dbs

                carry_box = [None]   # previous stream awaiting drain

                def run_stream(ic, hp, filler, jts_left):
                    jmax = 4 * ic + 4
                    qoffc = ic * 512
                    qoff = hp * S
                    pav = [avps.tile([128, 512], F32, name=f"av{e}",
                                     tag=f"av{e}") for e in range(2)]
                    pipe = []   # (jt, probs tile, c0), AV runs 2 jts behind
                    for jt in range(jmax):
                        jts_left[0] -= 1
                        r = jt - 4 * ic
                        c0 = 128 * r if r > 0 else 0
                        ps = scps.tile([128, 1024], F32, tag="scps")
                        for e in range(2):
                            psl = slice(64 * e, 64 * (e + 1))
                            nc.tensor.matmul(
                                ps[:, e * 512 + c0:(e + 1) * 512],
                                kT_sb[psl, qoff + jt * 128: qoff + (jt + 1) * 128],
                                qT_sb[psl, qoff + qoffc + c0: qoff + qoffc + 512],
                                start=True, stop=True)
                        drained = False
                        if jt == 1 and carry_box[0] is not None:
                            prev = carry_box[0]
                            carry_box[0] = None
                            drain(prev)
                            drained = True
                            if prev[1] == 1:   # chunk prev[0] fully done
                                pending.extend(make_units(prev[0]))
                        elif len(pipe) >= 2:
                            emit_av(hp, pav, *pipe.pop(0), stop=False)
                        # PE filler (qkv pair-1 closures in hp0, projection
                        # units in hp1) keeps the PE ramped while Act paces
                        # the exp pipeline; pop a second closure when the
                        # backlog exceeds the remaining slots
                        if not drained and filler:
                            filler.pop(0)()
                            if filler and len(filler) > jts_left[0]:
                                filler.pop(0)()
                        p = probsp.tile([128, 1024], F16, tag="p")
                        p3 = p[:].rearrange("p (b c) -> p b c", b=2)
                        ps3 = ps[:].rearrange("p (b c) -> p b c", b=2)
                        nc.scalar.activation(
                            p3[:, :, c0:512], ps3[:, :, c0:512],
                            mybir.ActivationFunctionType.Exp,
                            scale=SCALE)
                        if r >= 0:
                            nc.vector.tensor_mul(
                                p3[:, :, c0:c0 + 128],
                                p3[:, :, c0:c0 + 128],
                                mask3[:, :, 0:128])
                        pipe.append((jt, p, c0))
                    carry_box[0] = (ic, hp, pav, pipe)

                # ---- hp=0 phase with qkv pair-1 + v filler ----
                with (
                    tc.tile_pool(name="fp2", bufs=1, space="PSUM") as fp2,
                    tc.tile_pool(name="qpre2", bufs=3) as qprep2,
                    tc.tile_pool(name="ropet2", bufs=2) as ropetp2,
                ):
                    chain_state = {}

                    def mk_qk(mt, n):
                        def a(mt=mt, n=n):
                            pt = fp2.tile([128, 512], F32, name="fpt",
                                          tag="fpt")
                            chain_state[(mt, n)] = pt
                            for k in range(4):
                                nc.tensor.matmul(
                                    pt[:],
                                    w_sb[:, k * WF + mt * 128: k * WF + (mt + 1) * 128],
                                    xT_sb[:, k * S + n * 512: k * S + (n + 1) * 512],
                                    start=(k == 0), stop=False)

                        def b(mt=mt, n=n):
                            pt = chain_state.pop((mt, n))
                            for k in range(4, KT):
                                nc.tensor.matmul(
                                    pt[:],
                                    w_sb[:, k * WF + mt * 128: k * WF + (mt + 1) * 128],
                                    xT_sb[:, k * S + n * 512: k * S + (n + 1) * 512],
                                    start=False, stop=(k == KT - 1))
                            qpre = qprep2.tile([128, 512], F16, name="qpre2",
                                               tag="qpre2")
                            nc.vector.tensor_copy(qpre[:], pt[:])
                            dest = qT_sb if mt == 1 else kT_sb
                            rope_entries[(mt, n)] = (dest, S, n, qpre)

                        def r(mt=mt, n=n):
                            flush_rope((mt, n), fp2, "fpt", ropetp2,
                                       gps=True)
                        return [a, b, r]

                    def mk_v(st):
                        def vcl(st=st):
                            pv = fp2.tile([128, 512], F32, name="fpt",
                                          tag="fpt")
                            for k in range(KT):
                                nc.tensor.matmul(
                                    pv[:, 0:256],
                                    xT_sb[:, k * S + st * 128: k * S + (st + 1) * 128],
                                    w_sb[:, k * WF + VOFF: k * WF + WF],
                                    start=(k == 0), stop=(k == KT - 1))
                            vdst = v_sb[:, st * 260:(st + 1) * 260].rearrange(
                                "p (h c) -> p h c", c=65)[:, :, 0:64]
                            nc.vector.tensor_copy(
                                vdst, pv[:, 0:256].rearrange(
                                    "p (h c) -> p h c", c=64))
                        return vcl

                    qk = {(mt, n): mk_qk(mt, n)
                          for mt in (1, 3) for n in range(NC_CH)}
                    vv = [mk_v(st) for st in range(ST)]

                    def mk_r0(key):
                        def r0(key=key):
                            flush_rope(key, fp2, "fpt", ropetp2, gps=True)
                        return r0

                    fill = ([mk_r0((2, 1)), mk_r0((0, 0))]
                            + vv[0:4] + [mk_r0((2, 2))]
                            + vv[4:8] + [mk_r0((2, 3))]
                            + qk[(1, 1)] + qk[(3, 0)]
                            + [mk_r0((0, 3))]
                            + qk[(3, 1)] + vv[8:10]
                            + [mk_r0((0, 2))]
                            + qk[(1, 0)]
                            + vv[10:16]
                            + qk[(3, 2)] + qk[(3, 3)]
                            + qk[(1, 3)] + qk[(1, 2)])

                    jts0 = [sum(4 * ic + 4 for ic in CHUNK_ORDER)]
                    for ic in CHUNK_ORDER:
                        run_stream(ic, 0, fill, jts0)
                    # leftovers (late pair-1 ropes) drain into the Act
                    # exp backlog of the last hp0 stream
                    while fill:
                        fill.pop(0)()

                # ---- hp=1 phase with projection-unit filler ----
                with tc.tile_pool(name="yps", bufs=2, space="PSUM") as yps:
                    yps_ref["pool"] = yps
                    jts1 = [sum(4 * ic + 4 for ic in CHUNK_ORDER)]
                    for ic in CHUNK_ORDER:
                        run_stream(ic, 1, pending, jts1)

                    # tail: drain the last stream with a HAM keep-warm
                    # dummy (keyed on the drain's e=0 broadcast so it fires
                    # mid-drain), then its chunk's projection
                    dbs = drain(carry_box[0])
                    psd = scps.tile([128, 1024], F32, tag="scps")
                    nc.tensor.matmul(psd[0:64, 0:64], dbs[0][0:64, 0:64],
                                     dbs[0][0:64, 0:64], start=True, stop=True)
                    while pending:
                        pending.pop(0)()
                    for fn in make_units(CHUNK_ORDER[-1], last=True):
                        fn()
                    import os
                    if os.environ.get("DEBUG_DUMP") == "1":
                        # overwrite y rows with raw qT/kT/v for inspection
                        qv = qT_sb[:].rearrange("p (a c) -> p a c", c=1024)
                        kv = kT_sb[:].rearrange("p (a c) -> p a c", c=1024)
                        for a in range(4):
                            nc.sync.dma_start(
                                y[a * 128:(a + 1) * 128, :], qv[:, a, :])
                            nc.sync.dma_start(
                                y[512 + a * 128: 512 + (a + 1) * 128, :],
                                kv[:, a, :])
                        vv4 = v_sb[:, 0:4096].rearrange(
                            "p (a c) -> p a c", c=1024)
                        for a in range(4):
                            nc.sync.dma_start(
                                y[1024 + a * 128:1024 + (a + 1) * 128, :],
                                vv4[:, a, :])
                        xv = xT_sb[:, 0:4096].rearrange(
                            "p (a c) -> p a c", c=1024)
                        for a in range(4):
                            nc.sync.dma_start(
                                y[1536 + a * 128:1536 + (a + 1) * 128, :],
                                xv[:, a, :])

    nc.compile()
    return nc


def _rope_tables():
    inv_freq = 1.0 / (ROPE_BASE ** (np.arange(0, HD, 2, dtype=np.float64) / HD))
    t = np.arange(S, dtype=np.float64)
    freqs = np.outer(t, inv_freq)                      # [S, hd/2]
    emb = np.concatenate([freqs, freqs], axis=-1)      # [S, hd]
    cosT = np.cos(emb).T.astype(np.float32)            # [hd, S]
    sinT = np.sin(emb).T.astype(np.float32)
    cos2 = np.vstack([cosT, cosT])                     # [128, S]
    sin2 = np.vstack([sinT, sinT])
    return np.ascontiguousarray(cos2), np.ascontiguousarray(sin2)


def _rot_matrix():
    r = np.zeros((HD, HD), dtype=np.float32)
    half = HD // 2
    for d in range(half):
        r[d, d + half] = -1.0       # rot(q)[0:32] = -q[32:64]
        r[d + half, d] = 1.0        # rot(q)[32:64] = q[0:32]
    r2 = np.zeros((128, 128), dtype=np.float32)
    r2[0:HD, 0:HD] = r
    r2[HD:128, HD:128] = r
    return np.ascontiguousarray(r2.T)


def _mask_tile():
    # [128, 256]: the same lower-triangle-of-the-diagonal-128-block twice
    # (so a [128, 2, 128] view multiplies both heads of a pair at once)
    jl = np.arange(128)[:, None]
    il = np.arange(128)[None, :]
    tri = (jl <= il).astype(np.float32)
    return np.ascontiguousarray(np.concatenate([tri, tri], axis=1))


def _tile_rows(a):
    """[K*128, C] -> [128, K*C] with row r of tile k at partition r%...:
    a[k*128 + p, :] lands at [p, k*C : (k+1)*C]."""
    kk = a.shape[0] // 128
    return np.ascontiguousarray(
        a.reshape(kk, 128, a.shape[1]).transpose(1, 0, 2).reshape(128, -1))


_prog_cache = {}

# test harness hooks: set TRACE=True before calling kernel() to capture an
# NTFF profile; the BassKernelResults lands in LAST_RESULTS.
TRACE = False
LAST_RESULTS = None


def _f16(a):
    return np.ascontiguousarray(a.astype(np.float16))


def kernel(x, w_qkv, w_out, mask):
    x = np.asarray(x, dtype=np.float32)
    w_qkv = np.asarray(w_qkv, dtype=np.float32)
    w_out = np.asarray(w_out, dtype=np.float32)

    if "nc" not in _prog_cache:
        _prog_cache["nc"] = _build_program()
    nc = _prog_cache["nc"]

    cos2, sin2 = _rope_tables()
    rmatT = _rot_matrix()
    mask2 = _mask_tile()

    in_maps = []
    for c in range(N_CORES):
        b = c // 4
        g = c % 4
        cw = HEADS_PER_CORE * HD   # 256
        wq = w_qkv[:, g * cw:(g + 1) * cw]
        wk = w_qkv[:, D + g * cw: D + (g + 1) * cw]
        wv = w_qkv[:, 2 * D + g * cw: 2 * D + (g + 1) * cw]
        w_c = np.concatenate([wq, wk, wv], axis=1)
        wo_c = w_out[g * cw:(g + 1) * cw, :]
        xT_c = x[b].T
        in_maps.append({
            "xT": _f16(_tile_rows(xT_c)), "w": _f16(_tile_rows(w_c)),
            "wo": _f16(_tile_rows(wo_c)),
            "cosT": _f16(cos2), "sinT": _f16(sin2),
            "rmatT": _f16(rmatT), "mask2": _f16(mask2),
        })

    res = run_bass_kernel_spmd(nc, in_maps, list(range(N_CORES)),
                               trace=TRACE)
    global LAST_RESULTS
    LAST_RESULTS = res
    y = np.zeros((B, S, D), dtype=np.float32)
    for c in range(N_CORES):
        y[c // 4] += res.results[c]["y"].astype(np.float32)
    return y
